# revision 29
# baseline (speedup 1.0000x reference)
"""Sparse (sliding-window) attention head on 8 TRN2 NeuronCores.

Reference computation (B=2, S=4096, D=512, HD=64, SCALE=128):
    q = x @ wq ; k = x @ wk ; v = x @ wv          [B,S,64]
    scores[b,s,w] = q[b,s] . k[b,s-128+w] / 8     w in [0,256), zero-padded OOB
    out = softmax_w(scores) @ v_window            [B,S,64]

Sharding: 8 shards = (batch b, 1024-seq chunk c). Each shard gets a
zero-padded 128-halo of x on both sides, which reproduces the reference's
zero-padded (not masked) window semantics exactly. All compute is local,
no collectives.

Schedule (input-DMA-bound: ~1.5MB over ~240GB/s shared by 16 DMA engines;
the shard is processed in two sequence REGIONS so region-A work overlaps
region-B's input transfer):
  region A = padded cols 0:768  -> query blocks 0-3 (self-contained)
  region B = padded cols 768:1280 -> query blocks 4-7 (keys 512:1280)
  - 9 input DMAs byte-balanced across both HWDGE engines (sync+scalar),
    region-A pieces first in both queues; contiguous per-region host slabs
  - PE warmup matmuls under the DMA window hold the HAM clock gate at
    2.4 GHz; exp-pinned fillers keep it warm through the attention phase
  - band mask generated on-device (gpsimd iota + compares)
  - packed [wq|wk] projection per region (M=128), PSUM-accumulated over
    d-chunks; region-B qk hoisted between the v-chunk groups because its
    evac gates every score matmul via the PE's hoisted aggregate wait
  - v natural-layout, kc-outer (PSUM start=True clears the whole bank's
    has_written bits, so accumulation groups must be sequential per bank)
  - evacuations split across engines by PSUM bank: ACT takes region-A q/k
    (bank-split with DVE) plus both vaug copies; DVE takes region-B q/k
  - attention in 2 groups of 4 blocks: 12 score matmuls into a 3-bank
    PSUM tile, one big exp per group, masked multiply of the two
    triangular side chunks only (middle chunk of the band is always
    valid); softmax denominator via a ones-column in v
  - batched finale (reciprocal + broadcast multiply) per group, two
    contiguous output DMAs so group 0's store overlaps group 1's compute
"""

import sys
import types

import numpy as np
import ml_dtypes

B, S, D = 2, 4096, 512
HD = 64
SCALE = 128
SS = S // 4          # 1024 positions per shard
HP = SCALE           # halo padding each side
NP = SS + 2 * HP     # 1280 padded positions
NDC = D // 128       # 4 d-chunks
CA = 768             # region A cols (chunks 0-5, query blocks 0-3)
CB = NP - CA         # region B cols 768:1280 (chunks 6-9, blocks 4-7)

WARMUP = 7           # PE warmup matmuls under the DMA window (HAM un-throttle)

_CACHE = {}


def _ensure_hooks():
    """Register the axon NTFF profile hook; keep artifacts local."""
    if "antenv.axon_hooks" not in sys.modules:
        try:
            from trn_agent_boot.trn_boot import _ntff_profile_via_ctypes

            m = types.ModuleType("antenv.axon_hooks")
            m.get_axon_ntff_profile_hook = lambda: _ntff_profile_via_ctypes(
                "/opt/axon/libaxon_pjrt.so"
            )
            sys.modules["antenv.axon_hooks"] = m
        except Exception:
            pass
    import concourse.bass_utils as bass_utils

    bass_utils.upload_artifacts = lambda tmpdir: tmpdir


def _build_nc():
    import concourse.mybir as mybir
    import concourse.tile as tile
    from concourse import bacc

    bf = mybir.dt.bfloat16
    f32 = mybir.dt.float32
    AF = mybir.ActivationFunctionType
    AL = mybir.AluOpType

    nc = bacc.Bacc("TRN2", target_bir_lowering=False, debug=False, num_devices=8)

    wm_d = nc.dram_tensor("wm", [128, 6, 128], bf, kind="ExternalInput")
    xa_d = nc.dram_tensor("xTa", [D, CA], bf, kind="ExternalInput")
    xb_d = nc.dram_tensor("xTb", [D, CB], bf, kind="ExternalInput")
    outA_d = nc.dram_tensor("outA", [128, 4, HD], bf, kind="ExternalOutput")
    outB_d = nc.dram_tensor("outB", [128, 4, HD], bf, kind="ExternalOutput")

    with tile.TileContext(nc) as tc:
        with (
            tc.tile_pool(name="consts", bufs=1) as consts,
            tc.tile_pool(name="xtp", bufs=1) as xtp,
            tc.tile_pool(name="qkp", bufs=1) as qkp,
            tc.tile_pool(name="vgp", bufs=1) as vgp,
            tc.tile_pool(name="exp", bufs=2) as expool,
            tc.tile_pool(name="emp", bufs=2) as empool,
            tc.tile_pool(name="fin", bufs=1) as fin,
        ):
            wm_s = consts.tile([128, 6, 128], bf)
            xa = [xtp.tile([128, CA], bf, tag=f"xa{dc}", name=f"xa{dc}") for dc in range(NDC)]
            xb = [xtp.tile([128, CB], bf, tag=f"xb{dc}", name=f"xb{dc}") for dc in range(NDC)]

            # Input DMAs split across both HWDGE engines; all region-A
            # pieces issued first on both queues so region A lands early.
            nc.sync.dma_start(out=wm_s, in_=wm_d[:, :, :])
            nc.scalar.dma_start(out=xa[0], in_=xa_d[0:128, :])
            nc.sync.dma_start(out=xa[1], in_=xa_d[128:256, :])
            nc.scalar.dma_start(out=xa[2], in_=xa_d[256:384, :])
            nc.sync.dma_start(out=xa[3], in_=xa_d[384:512, :])
            nc.scalar.dma_start(out=xb[0], in_=xb_d[0:128, :])
            nc.scalar.dma_start(out=xb[1], in_=xb_d[128:256, :])
            nc.scalar.dma_start(out=xb[2], in_=xb_d[256:384, :])
            nc.sync.dma_start(out=xb[3], in_=xb_d[384:512, :])

            # Trigger the ACT exp table load now (1.3us) so it hides
            # under the transfers.
            zz = consts.tile([128, 1], f32)
            nc.vector.memset(zz, 0.0)
            ez = consts.tile([128, 1], f32)
            nc.scalar.activation(ez, zz, AF.Exp)

            # Band mask built on-device (DVE is idle during the input
            # window): iot[p, q] = q - p, mask0 = (p >= q), mask1 = (p < q).
            i32 = mybir.dt.int32
            iot = consts.tile([128, 128], i32)
            nc.gpsimd.iota(iot, [[1, 128]], base=0, channel_multiplier=-1)
            mk = consts.tile([128, 2, 128], bf)
            nc.gpsimd.tensor_scalar(out=mk[:, 0, :], in0=iot, scalar1=0,
                                    scalar2=None, op0=AL.is_le)
            nc.gpsimd.tensor_scalar(out=mk[:, 1, :], in0=iot, scalar1=0,
                                    scalar2=None, op0=AL.is_gt)

            # DMA-free garbage tile for PE warmup.
            garb = consts.tile([128, 512], bf)
            nc.vector.memset(garb, 0.5)

            kTa = qkp.tile([64, CA], bf, tag="kTa")
            qTa = qkp.tile([64, 640], bf, tag="qTa")
            kTb = qkp.tile([64, CB], bf, tag="kTb")
            qTb = qkp.tile([64, 384], bf, tag="qTb")
            vaugA = vgp.tile([128, 6, 66], bf, tag="vaugA")
            nc.vector.memset(vaugA[:, :, 64:66], 1.0)
            vaugB = vgp.tile([128, 4, 66], bf, tag="vaugB")
            nc.vector.memset(vaugB[:, :, 64:66], 1.0)

            outsb = fin.tile([128, 8, HD], bf)
            rc = fin.tile([128, 8, 1], f32)

            def k_chunk(c):
                return kTa[:, c * 128 : (c + 1) * 128] if c < 6 else \
                    kTb[:, (c - 6) * 128 : (c - 5) * 128]

            def q_block(qb):
                return qTa[:, qb * 128 : (qb + 1) * 128] if qb < 5 else \
                    qTb[:, (qb - 5) * 128 : (qb - 4) * 128]

            def v_chunk(c):
                return vaugA[:, c, 0:65] if c < 6 else vaugB[:, c - 6, 0:65]

            with (
                tc.tile_pool(name="aps1", bufs=1, space="PSUM") as aps1,
            ):
                    # Allocation order matters: the attention sc tiles
                    # reuse these banks first-fit, so put the banks whose
                    # readers finish earliest (warm, qksA, vpsA) first.
                    wps = aps1.tile([128, 512], f32, tag="warm")
                    qksA0 = aps1.tile([128, 768], f32, tag="qksA")
                    vpsA0 = aps1.tile([128, 6, HD], f32, tag="vpsA")
                    qksB = aps1.tile([128, 512], f32, tag="qksB")
                    vpsB = aps1.tile([128, 4, HD], f32, tag="vpsB")
                    for _ in range(WARMUP):
                        nc.tensor.matmul(
                            wps, lhsT=garb[:, 0:128], rhs=garb, start=True, stop=True
                        )

                    qksA, vpsA = qksA0, vpsA0

                    # Region A: packed q|k projection + v, d-chunk outer
                    # so matmuls start as each xa DMA lands.
                    for dc in range(NDC):
                        nc.tensor.matmul(
                            qksA[:, 0:512],
                            lhsT=wm_s[:, dc, :],
                            rhs=xa[dc][:, 0:512],
                            start=(dc == 0),
                            stop=(dc == NDC - 1),
                        )
                        nc.tensor.matmul(
                            qksA[:, 512:768],
                            lhsT=wm_s[:, dc, :],
                            rhs=xa[dc][:, 512:768],
                            start=(dc == 0),
                            stop=(dc == NDC - 1),
                        )
                    # v accumulation groups must be sequential per PSUM
                    # bank (start=True clears has_written for the WHOLE
                    # bank), so kc-outer / dc-inner. First half of vA, then
                    # region-B qk (its evac is the hoisted gate for every
                    # score matmul), then the rest of v.
                    for kc in range(3):
                        for dc in range(NDC):
                            nc.tensor.matmul(
                                vpsA[:, kc, :],
                                lhsT=xa[dc][:, kc * 128 : (kc + 1) * 128],
                                rhs=wm_s[:, 4 + dc // 2, (dc % 2) * 64 : (dc % 2) * 64 + 64],
                                start=(dc == 0),
                                stop=(dc == NDC - 1),
                            )
                    for dc in range(NDC):
                        nc.tensor.matmul(
                            qksB,
                            lhsT=wm_s[:, dc, :],
                            rhs=xb[dc],
                            start=(dc == 0),
                            stop=(dc == NDC - 1),
                        )
                    for kc in range(3, 6):
                        for dc in range(NDC):
                            nc.tensor.matmul(
                                vpsA[:, kc, :],
                                lhsT=xa[dc][:, kc * 128 : (kc + 1) * 128],
                                rhs=wm_s[:, 4 + dc // 2, (dc % 2) * 64 : (dc % 2) * 64 + 64],
                                start=(dc == 0),
                                stop=(dc == NDC - 1),
                            )
                    for kc in range(4):
                        for dc in range(NDC):
                            nc.tensor.matmul(
                                vpsB[:, kc, :],
                                lhsT=xb[dc][:, kc * 128 : (kc + 1) * 128],
                                rhs=wm_s[:, 4 + dc // 2, (dc % 2) * 64 : (dc % 2) * 64 + 64],
                                start=(dc == 0),
                                stop=(dc == NDC - 1),
                            )

                    # Region A q/k evac on ACT (same PSUM banks -> one
                    # engine); B q/k evac FIRST on DVE (hoisted score gate),
                    # vaug evacs after.
                    # Region-A evacs split by PSUM bank: ACT reads bank 1
                    # (cols 0:512) while DVE reads bank 2 (cols 512:768).
                    nc.scalar.copy(kTa[:, 0:512], qksA[64:128, 0:512])
                    nc.vector.tensor_copy(kTa[:, 512:768], qksA[64:128, 512:768])
                    nc.scalar.copy(qTa[:, 0:384], qksA[0:64, 128:512])
                    nc.vector.tensor_copy(qTa[:, 384:640], qksA[0:64, 512:768])
                    nc.vector.tensor_copy(kTb, qksB[64:128, :])
                    nc.vector.tensor_copy(qTb, qksB[0:64, 0:384])
                    nc.scalar.copy(vaugA[:, :, 0:64], vpsA)
                    nc.scalar.copy(vaugB[:, :, 0:64], vpsB)

            # All projection PSUM (6 banks) freed here; attention uses
            # sc (2x3 banks) + av (2x1 bank) = 8.
            with (
                tc.tile_pool(name="sps0", bufs=1, space="PSUM") as sps0,
                tc.tile_pool(name="sps1", bufs=1, space="PSUM") as sps1,
                tc.tile_pool(name="avp", bufs=1, space="PSUM") as avp,
            ):
                    mask_b = mk[:, :, :].unsqueeze(1).broadcast_to([128, 4, 2, 128])
                    scs, exs, ems = [], [], []
                    for g in range(2):
                        sc4 = (sps0 if g == 0 else sps1).tile(
                            [128, 4, 3, 128], f32, tag=f"sc{g}", name=f"sc{g}")
                        for b in range(4):
                            qb = g * 4 + b
                            for c in range(3):
                                nc.tensor.matmul(
                                    sc4[:, b, c, :],
                                    lhsT=k_chunk(qb + c),
                                    rhs=q_block(qb),
                                    start=True,
                                    stop=True,
                                )
                        ex4 = expool.tile([128, 4, 3, 128], bf, tag="ex", name=f"ex{g}")
                        nc.scalar.activation(ex4, sc4, AF.Exp, scale=0.125)
                        em4 = empool.tile([128, 4, 2, 128], bf, tag="em", name=f"em{g}")
                        nc.vector.tensor_tensor(
                            out=em4, in0=ex4[:, :, 0:3:2, :], in1=mask_b, op=AL.mult
                        )
                        for f in range(3):
                            nc.tensor.matmul(
                                sc4[:, 0, :, :],
                                lhsT=garb[:, 0:128],
                                rhs=ex4[:, f, :, :],
                                start=True,
                                stop=True,
                            )
                        if g == 0:
                            # two more fillers pinned on em4 cover the
                            # 19.5-20.6us PE-idle window so HAM keeps the
                            # clock at 2.4 GHz for the group-1 AV matmuls
                            for f in range(2):
                                nc.tensor.matmul(
                                    sc4[:, 1, 0:2, :],
                                    lhsT=garb[:, 0:128],
                                    rhs=em4[:, f, :, :],
                                    start=True,
                                    stop=True,
                                )
                        scs.append(sc4)
                        exs.append(ex4)
                        ems.append(em4)

                    for g in range(2):
                        sc4, ex4, em4 = scs[g], exs[g], ems[g]
                        av = avp.tile([128, 4, 65], f32, tag=f"av{g}", name=f"av{g}")
                        for b in range(4):
                            qb = g * 4 + b
                            for c in range(3):
                                lhsT = (
                                    em4[:, b, 0, :]
                                    if c == 0
                                    else (ex4[:, b, 1, :] if c == 1 else em4[:, b, 1, :])
                                )
                                nc.tensor.matmul(
                                    av[:, b, 0:65],
                                    lhsT=lhsT,
                                    rhs=v_chunk(qb + c),
                                    start=(c == 0),
                                    stop=(c == 2),
                                )
                        sl = slice(g * 4, (g + 1) * 4)
                        nc.vector.reciprocal(rc[:, sl, :], av[:, :, 64:65])
                        nc.vector.tensor_tensor(
                            out=outsb[:, sl, :],
                            in0=av[:, :, 0:64],
                            in1=rc[:, sl, :].broadcast_to([128, 4, HD]),
                            op=AL.mult,
                        )
                        nc.sync.dma_start(
                            out=(outA_d if g == 0 else outB_d)[:, :, :],
                            in_=outsb[:, sl, :],
                        )

    nc.compile()
    return nc


def _get_nc():
    if "nc" not in _CACHE:
        _ensure_hooks()
        _CACHE["nc"] = _build_nc()
    return _CACHE["nc"]


def _host_inputs(inputs, wq, wk, wv):
    bf16 = ml_dtypes.bfloat16
    x = np.asarray(inputs, dtype=np.float32)
    wq = np.asarray(wq, dtype=np.float32)
    wk = np.asarray(wk, dtype=np.float32)
    wv = np.asarray(wv, dtype=np.float32)

    wm = np.zeros((128, 6, 128), np.float32)
    for dc in range(NDC):
        wm[:, dc, 0:64] = wq[dc * 128 : (dc + 1) * 128, :]
        wm[:, dc, 64:128] = wk[dc * 128 : (dc + 1) * 128, :]
    for dc in range(NDC):
        wm[:, 4 + dc // 2, (dc % 2) * 64 : (dc % 2) * 64 + 64] = (
            wv[dc * 128 : (dc + 1) * 128, :]
        )
    wm = wm.astype(bf16)

    in_maps = []
    for i in range(8):
        b, c = divmod(i, 4)
        s0 = c * SS
        xp = np.zeros((NP, D), np.float32)
        lo = max(0, s0 - HP)
        hi = min(S, s0 + SS + HP)
        xp[lo - (s0 - HP) : hi - (s0 - HP)] = x[b, lo:hi]
        xT = xp.T.astype(bf16)                               # [512, 1280]
        xTa = np.ascontiguousarray(xT[:, 0:CA])
        xTb = np.ascontiguousarray(xT[:, CA:NP])
        in_maps.append({"wm": wm, "xTa": xTa, "xTb": xTb})
    return in_maps


def run_sharded(inputs, wq, wk, wv, trace=False, trace_cores=None):
    """Run the SPMD kernel; returns (out [B,S,HD] f32, BassKernelResults)."""
    _ensure_hooks()
    import concourse.bass_utils as bass_utils

    nc = _get_nc()
    in_maps = _host_inputs(inputs, wq, wk, wv)
    res = bass_utils.run_bass_kernel_spmd(
        nc,
        in_maps,
        core_ids=list(range(8)),
        trace=trace,
        trace_cores=trace_cores,
    )
    out = np.empty((B, S, HD), np.float32)
    for i in range(8):
        b, c = divmod(i, 4)
        oa = np.asarray(res.results[i]["outA"]).astype(np.float32)
        ob = np.asarray(res.results[i]["outB"]).astype(np.float32)
        o = np.concatenate([oa, ob], axis=1)                 # [128, 8, 64]
        out[b, c * SS : (c + 1) * SS] = o.transpose(1, 0, 2).reshape(SS, HD)
    return out, res


def kernel(inputs, wq, wk, wv):
    out, _ = run_sharded(inputs, wq, wk, wv, trace=False)
    return out
